# revision 43
# baseline (speedup 1.0000x reference)
"""Trainium2 Bass kernel for the DDF (dynamic-filter + ECA + BN) module.

Distribution: data-parallel over batch B=8 across 8 NeuronCores (one image
per core).  All parameters replicated.  BN batch stats are all-reduced
across cores (sync-BN semantics, matching the reference).

Per-core layout: channels on partitions (2 channel-tiles of 128), pixels on
the free dimension.  The per-pixel filter generator (1x1 conv C -> C*9) is
permuted on the host so each PE output m-tile is one (tap, channel-tile)
pair, in the order taps are consumed.

x is shipped ONCE in a 66x66 zero-padded layout (one pad column each side
of every row, one zero guard row top/bottom), so every 3x3 tap window —
including the column-shifted ones — is just an offset strided-AP view of
the same buffer.  This cuts input HBM traffic ~3x vs shipping three
shifted copies (the prologue is HBM-bandwidth-bound with all 8 cores
loading at once) and removes all side-buffer DMA scheduling.

Engine split per chunk: mm1 PSUM evictions (+bias, fp32->bf16) on the
scalar engine, tap products as single paired [128, 2, 16, 64] DVE ops,
the fused-sum add tree on the DVE with the earliest-ready products joining
the mm2 contraction directly, BN stats straight from PSUM.  A dummy
all-reduce at kernel start prepays the ~15 us CC-core collective setup.
"""

import os

import numpy as np
import ml_dtypes

import concourse.bass as bass
import concourse.mybir as mybir
import concourse.tile as tile
from concourse import bacc
from concourse.bass_utils import run_bass_kernel_spmd

B, C, H, W = 8, 256, 64, 64
KS = 3
HW = H * W                    # 4096
XROW = W + 2                  # 66: one zero pad column on each side
XNR = H + 2                   # 66: one zero guard row top and bottom
XBUF = XROW * XNR             # 4356 padded pixels per channel
NCORES = 8
CT = 2                        # channel tiles of 128
MT1 = KS * KS * CT            # 18 mm1 output m-tiles
BN_EPS = 1e-5
F32 = mybir.dt.float32
BF16 = mybir.dt.bfloat16
ROWS_PER_CHUNK = 16
NCHUNKS = H // ROWS_PER_CHUNK  # 4
CHUNK = ROWS_PER_CHUNK * W     # 1024 pixels per chunk per channel-tile
NH = CHUNK // 512              # 512-px matmul groups per chunk

AF = mybir.ActivationFunctionType
ALU = mybir.AluOpType

# Tap order within a chunk: center-window taps (dj==1) first so chunk-0
# compute starts before the side buffers arrive; then left taps, then right.
# All taps go scalar-engine evict (+bias) -> bf16 DVE product: GPSIMD cannot
# read PSUM and its SW elementwise path is ~10x slower than the DVE (and
# poisons DVE throughput via SBUF port contention), so only the scalar and
# vector engines carry the eviction+product work.
TAPORD = [1, 4, 7, 0, 2, 3, 6, 5, 8]


def _emit(tc):
    nc = tc.nc

    # x in padded layout: [ct, c, (row+1)*66 + col + 1], zeros in the pads
    xp = nc.declare_dram_parameter("xp", [CT, 128, XBUF], BF16, isOutput=False)
    wf = nc.declare_dram_parameter("wf", [CT, 128, MT1 * 128], BF16, isOutput=False)
    bfp = nc.declare_dram_parameter("bfp", [128, MT1], F32, isOutput=False)
    wp = nc.declare_dram_parameter("wp", [CT, 128, C], BF16, isOutput=False)
    weca = nc.declare_dram_parameter("weca", [1, 3], F32, isOutput=False)
    gam = nc.declare_dram_parameter("gam", [128, CT], F32, isOutput=False)
    bet = nc.declare_dram_parameter("bet", [128, CT], F32, isOutput=False)
    yout = nc.declare_dram_parameter("y", [CT, 128, HW], F32, isOutput=True)

    with (
        tc.tile_pool(name="consts", bufs=1) as consts,
        tc.tile_pool(name="fps", bufs=3, space="PSUM") as fps,
        tc.tile_pool(name="yps", bufs=2, space="PSUM") as yps,
        tc.tile_pool(name="fsb", bufs=4) as fsb_pool,
        tc.tile_pool(name="prod", bufs=2) as prod_pool,
        tc.tile_pool(name="dram", bufs=1, space="DRAM") as dram,
    ):
        # ---- resident tensors -------------------------------------------
        wf_sb = [consts.tile([128, MT1 * 128], BF16, tag=f"wf{kt}", name=f"wf{kt}")
                 for kt in range(CT)]
        wp_sb = [consts.tile([128, C], BF16, tag=f"wp{kt}", name=f"wp{kt}")
                 for kt in range(CT)]
        bfp_sb = consts.tile([128, MT1], F32, tag="bfp", name="bfp")
        gam_sb = consts.tile([128, CT], F32, tag="gam", name="gam")
        bet_sb = consts.tile([128, CT], F32, tag="bet", name="bet")
        wecab = consts.tile([128, 3], F32, tag="wecab", name="wecab")
        # both channel-tiles of padded x in one tile, so a single strided-AP
        # DVE op can process both ct halves of a tap
        xp_sb = consts.tile([128, CT * XBUF], BF16, tag="xp", name="xp")
        y_sb = [consts.tile([128, HW], F32, tag=f"ysb{mt}", name=f"ysb{mt}")
                for mt in range(CT)]
        stats_sb = [
            consts.tile([128, NCHUNKS * NH, 6], F32, tag=f"st{mt}", name=f"st{mt}")
            for mt in range(CT)
        ]

        # ---- input DMA ---------------------------------------------------
        # All input flows in strict need order on two queues.  The prologue
        # is HBM-bandwidth bound (8 cores load concurrently; completions are
        # roughly fair-shared), so nothing non-critical may run early.
        def xp_dma(q, ct, r0, r1):
            """Padded rows r0..r1 (buffer row index, 0..66) of channel-tile ct."""
            q.dma_start(
                out=xp_sb[:, ct * XBUF + r0 * XROW : ct * XBUF + r1 * XROW],
                in_=xp[ct, :, r0 * XROW : r1 * XROW],
            )

        # wf is laid out in TAPORD order: slice A = tap positions 0-1,
        # B = 2-4, C = 5-6, D = 7-8 (columns of 128 per (pos, ct) tile).
        def wf_dma(q, kt, c0, c1):
            q.dma_start(out=wf_sb[kt][:, c0:c1], in_=wf[kt, :, c0:c1])

        # Everything flows on ONE queue in strict need order: the queue's
        # DMA ring (~4-5 in flight) completes roughly FIFO, so this is the
        # only way to guarantee the critical pieces finish first when all
        # 8 cores share HBM bandwidth.
        sp = nc.sync
        for kt in range(CT):
            wf_dma(sp, kt, 0, 512)              # slice A (pos 0-1)
        for ct in range(CT):
            xp_dma(sp, ct, 0, 10)               # image rows -1..8
        for ct in range(CT):
            xp_dma(sp, ct, 10, 18)              # image rows 9..16
        sp.dma_start(out=bfp_sb[:], in_=bfp[:, :])
        for kt in range(CT):
            wf_dma(sp, kt, 512, 1280)           # slice B (pos 2-4)
        for kt in range(CT):
            wf_dma(sp, kt, 1280, 1792)          # slice C (pos 5-6)
        for ct in range(CT):
            xp_dma(sp, ct, 18, 34)              # chunk 1
        for kt in range(CT):
            wf_dma(sp, kt, 1792, MT1 * 128)     # slice D (pos 7-8)
        for kt in range(CT):
            sp.dma_start(out=wp_sb[kt][:], in_=wp[kt])
        for ct in range(CT):
            xp_dma(sp, ct, 34, 50)              # chunk 2
        sp.dma_start(out=wecab[:], in_=weca[0:1, :].to_broadcast([128, 3]))
        for ct in range(CT):
            xp_dma(sp, ct, 50, XNR)             # chunk 3 (+ bottom guard)
        sp.dma_start(out=gam_sb[:], in_=gam[:, :])
        sp.dma_start(out=bet_sb[:], in_=bet[:, :])

        xpv = xp_sb.rearrange("p (t r c) -> p t r c", t=CT, c=XROW)

        def win_mm(kt, row0, nrows):
            """Center window rows row0..row0+nrows as a [128, nrows, 64]
            strided AP for the mm1 rhs (channel-tile kt)."""
            return xpv[:, kt, row0 + 1 : row0 + 1 + nrows, 1 : 1 + W]

        def win_ct(ct, row0, dj=1, nrows=ROWS_PER_CHUNK):
            """One channel-tile of a (dj-shifted) window, [128, nrows, 64]."""
            return xpv[:, ct, row0 + 1 : row0 + 1 + nrows, dj : dj + W]

        def win2(dj, row0, nrows=ROWS_PER_CHUNK):
            """Both channel-tiles of a dj-shifted window as one
            [128, 2, nrows, 64] AP (pads supply the shifted-in zeros)."""
            return xpv[:, :, row0 + 1 : row0 + 1 + nrows, dj : dj + W]

        # ---- warmup collective ------------------------------------------
        # The CC sidecar core takes ~15 us of setup between the trigger and
        # the start of the mesh algorithm.  Fire a dummy all-reduce at kernel
        # start so that setup (ring/channel init) overlaps the main loop
        # instead of sitting on the critical path of the BN-stats reduce.
        wrm = consts.tile([128, 1], F32, tag="wrm", name="wrm")
        nc.vector.memset(wrm[:], 0.0)
        wrm_in = dram.tile([128, 1], F32, tag="wrmi", name="wrmi")
        wrm_out = dram.tile([128, 1], F32, tag="wrmo", name="wrmo")
        nc.gpsimd.dma_start(out=wrm_in[:], in_=wrm[:])
        nc.gpsimd.collective_compute(
            "AllReduce",
            ALU.add,
            replica_groups=[list(range(NCORES))],
            ins=[wrm_in[:].opt()],
            outs=[wrm_out[:].opt()],
        )

        # ---- ECA state tiles (filled inside the chunk loop so no engine
        # queue stalls waiting for the full-image pooled sum) --------------
        poolp = consts.tile([128, CT, NCHUNKS], F32, tag="poolp", name="poolp")
        pool2 = consts.tile([128, CT], F32, tag="pool2", name="pool2")
        shd = consts.tile([128, CT], F32, tag="shd", name="shd")  # pooled[c-1]
        shu = consts.tile([128, CT], F32, tag="shu", name="shu")  # pooled[c+1]
        eca1 = consts.tile([128, CT], F32, tag="eca1", name="eca1")
        eca2 = consts.tile([128, CT], F32, tag="eca2", name="eca2")
        attn = consts.tile([128, CT], F32, tag="attn", name="attn")
        nc.vector.memset(shd[:], 0.0)
        nc.vector.memset(shu[:], 0.0)

        xpf = xp_sb.rearrange("p (t x) -> p t x", t=CT)

        def pool_reduce(ci):
            # flat padded span of the chunk's rows; the pad zeros are
            # harmless in the sum
            a = (1 + ROWS_PER_CHUNK * ci) * XROW
            b = a + ROWS_PER_CHUNK * XROW
            nc.vector.tensor_reduce(
                out=poolp[:, :, ci : ci + 1],
                in_=xpf[:, :, a:b],
                axis=mybir.AxisListType.X,
                op=ALU.add,
            )

        def eca_attn():
            """pooled -> attn. Channel shifts cross the two channel-tiles
            via tiny partition-offset DMAs (on the sync queue, which is done
            with its input DMAs by now)."""
            for ct in range(CT):
                nc.vector.tensor_reduce(
                    out=pool2[:, ct : ct + 1],
                    in_=poolp[:, ct, :],
                    axis=mybir.AxisListType.X,
                    op=ALU.add,
                )
            for ct in range(CT):
                nc.sync.dma_start(
                    out=shd[1:128, ct : ct + 1], in_=pool2[0:127, ct : ct + 1]
                )
                nc.sync.dma_start(
                    out=shu[0:127, ct : ct + 1], in_=pool2[1:128, ct : ct + 1]
                )
            nc.sync.dma_start(out=shd[0:1, 1:2], in_=pool2[127:128, 0:1])
            nc.sync.dma_start(out=shu[127:128, 0:1], in_=pool2[0:1, 1:2])
            nc.vector.tensor_scalar(
                out=eca1, in0=shd[:], scalar1=wecab[:, 0:1], scalar2=None,
                op0=ALU.mult,
            )
            nc.vector.scalar_tensor_tensor(
                out=eca2, in0=pool2[:], scalar=wecab[:, 1:2], in1=eca1[:],
                op0=ALU.mult, op1=ALU.add,
            )
            nc.vector.scalar_tensor_tensor(
                out=eca1, in0=shu[:], scalar=wecab[:, 2:3], in1=eca2[:],
                op0=ALU.mult, op1=ALU.add,
            )
            # attn = sigmoid(eca) = 1 / (1 + exp(-eca))
            nc.scalar.activation(out=eca2[:], in_=eca1[:], func=AF.Exp, scale=-1.0)
            nc.vector.tensor_scalar(
                out=attn, in0=eca2[:], scalar1=1.0, scalar2=None, op0=ALU.add
            )
            nc.vector.reciprocal(out=attn[:], in_=attn[:])

        def make_cfb(r0):
            cfb = prod_pool.tile([128, CT * CHUNK], BF16, tag="cf", name="cf")
            for ct in range(CT):
                dst = cfb[:, ct * CHUNK : (ct + 1) * CHUNK]
                nc.scalar.activation(
                    out=dst.rearrange("p (r c) -> p r c", c=W),
                    in_=win_ct(ct, r0), func=AF.Identity,
                    scale=attn[:, ct : ct + 1],
                )
            return cfb

        # ---- main loop over row chunks ----------------------------------
        for ci in range(NCHUNKS):
            r0 = ci * ROWS_PER_CHUNK
            prods = {}
            cfb = None
            for pos, k in enumerate(TAPORD):
                if ci == 0 and pos == 1:
                    pool_reduce(0)
                di, dj = divmod(k, KS)
                pr = prod_pool.tile([128, CT * CHUNK], BF16, tag=f"pr{k}",
                                    name=f"pr{k}")
                fsb = fsb_pool.tile([128, CT * CHUNK], BF16, tag="fsb",
                                    name="fsb")
                for ct in range(CT):
                    mt = pos * CT + ct
                    fp = fps.tile([128, CHUNK], F32, tag="fp", name="fp")
                    for nh in range(NH):
                        for kt in range(CT):
                            lhsT = wf_sb[kt][:, mt * 128 : (mt + 1) * 128]
                            rhs = win_mm(kt, r0 + nh * 8, 8)
                            nc.tensor.matmul(
                                fp[:, nh * 512 : (nh + 1) * 512],
                                lhsT,
                                rhs,
                                start=(kt == 0),
                                stop=(kt == CT - 1),
                            )
                    # scalar-engine evict (+bias, fp32->bf16)
                    nc.scalar.activation(
                        out=fsb[:, ct * CHUNK : (ct + 1) * CHUNK], in_=fp[:],
                        func=AF.Identity, bias=bfp_sb[:, mt : mt + 1],
                        scale=1.0,
                    )
                # one DVE product covers both channel-tiles via strided APs
                nc.vector.tensor_tensor(
                    out=pr.rearrange("p (t r c) -> p t r c", t=CT, c=W),
                    in0=fsb.rearrange("p (t r c) -> p t r c", t=CT, c=W),
                    in1=win2(dj, r0 + di - 1),
                    op=ALU.mult,
                )
                prods[k] = pr

                # interleave adds / attention work as results become available;
                # the tree is arranged so only ONE add remains after the last
                # tap's product (short chunk tail).
                if pos == 2:
                    if ci > 0:
                        cfb = make_cfb(r0)
                elif pos == 3:
                    if ci > 0:
                        nc.vector.tensor_add(prods[0][:], prods[0][:], cfb[:])
                    else:
                        pool_reduce(1)
                elif pos == 4:
                    nc.vector.tensor_add(prods[0][:], prods[0][:], prods[2][:])
                elif pos == 5:
                    nc.vector.tensor_add(prods[4][:], prods[4][:], prods[0][:])
                    if ci == 0:
                        pool_reduce(2)
                elif pos == 6:
                    nc.vector.tensor_add(prods[3][:], prods[3][:], prods[6][:])
                elif pos == 7:
                    if ci == 0:
                        # all center pieces have arrived by now; finish the
                        # pooled sum and the attention on otherwise-idle slack
                        pool_reduce(3)
                        eca_attn()
                        cfb = make_cfb(r0)
                    nc.vector.tensor_add(prods[3][:], prods[3][:], prods[5][:])
                    nc.vector.tensor_add(prods[4][:], prods[4][:], prods[3][:])
                elif pos == 8:
                    nc.vector.tensor_add(prods[4][:], prods[4][:], prods[8][:])

            # projection matmul + y eviction + local BN stats.
            # fused tree result lives in prods[4]; p7 and p1 (ready earliest)
            # join the contraction directly, trading PE cycles for DVE adds.
            # In chunk 0 the attention term joins the contraction too (it is
            # only ready late there).
            mm2_srcs = [prods[7], prods[1], prods[4]]
            if ci == 0:
                mm2_srcs.append(cfb)
            for mt2 in range(CT):
                ypt = [yps.tile([128, 512], F32, tag="yp", name="yp")
                       for _ in range(NH)]
                nsrc = len(mm2_srcs)
                for si, srct in enumerate(mm2_srcs):
                    for kt in range(CT):
                        lhsT2 = wp_sb[kt][:, mt2 * 128 : (mt2 + 1) * 128]
                        for nh in range(NH):
                            nc.tensor.matmul(
                                ypt[nh][:],
                                lhsT2,
                                srct[:, kt * CHUNK + nh * 512 : kt * CHUNK + (nh + 1) * 512],
                                start=(si == 0 and kt == 0),
                                stop=(si == nsrc - 1 and kt == CT - 1),
                            )
                for nh in range(NH):
                    dst = y_sb[mt2][:, r0 * W + nh * 512 : r0 * W + (nh + 1) * 512]
                    # stats straight from PSUM, in parallel with the evict
                    nc.vector.bn_stats(
                        out=stats_sb[mt2][:, ci * NH + nh, :], in_=ypt[nh][:]
                    )
                    nc.scalar.activation(out=dst, in_=ypt[nh][:], func=AF.Copy)

        # ---- global BN stats via all-reduce -----------------------------
        ps = consts.tile([128, 2 * CT], F32, tag="ps", name="ps")
        for mt2 in range(CT):
            mv = consts.tile([128, 2], F32, tag=f"mv{mt2}", name=f"mv{mt2}")
            nc.vector.bn_aggr(out=mv[:], in_=stats_sb[mt2][:])
            mean = mv[:, 0:1]
            var = mv[:, 1:2]
            nc.vector.tensor_scalar(
                out=ps[:, 2 * mt2 : 2 * mt2 + 1], in0=mean, scalar1=float(HW),
                scalar2=None, op0=ALU.mult,
            )
            # sumsq = (var + mean^2) * HW
            nc.vector.scalar_tensor_tensor(
                out=ps[:, 2 * mt2 + 1 : 2 * mt2 + 2], in0=mean, scalar=mean,
                in1=var, op0=ALU.mult, op1=ALU.add,
            )
            nc.vector.tensor_scalar(
                out=ps[:, 2 * mt2 + 1 : 2 * mt2 + 2],
                in0=ps[:, 2 * mt2 + 1 : 2 * mt2 + 2],
                scalar1=float(HW), scalar2=None, op0=ALU.mult,
            )

        ps_b = dram.tile([128, 2 * CT], F32, tag="psb", name="psb")
        gs_b = dram.tile([128, 2 * CT], F32, tag="gsb", name="gsb")
        nc.sync.dma_start(out=ps_b[:], in_=ps[:])
        nc.gpsimd.collective_compute(
            "AllReduce",
            ALU.add,
            replica_groups=[list(range(NCORES))],
            ins=[ps_b[:].opt()],
            outs=[gs_b[:].opt()],
        )
        gs = consts.tile([128, 2 * CT], F32, tag="gs", name="gs")
        nc.scalar.dma_start(out=gs[:], in_=gs_b[:])

        # ---- normalize and write out ------------------------------------
        minv = 1.0 / float(B * HW)
        NSL = 4  # normalize/store slices per channel-tile
        SL = HW // NSL
        mg = consts.tile([128, CT], F32, tag="mg", name="mg")
        vg = consts.tile([128, CT], F32, tag="vg", name="vg")
        rr = consts.tile([128, CT], F32, tag="rr", name="rr")
        tt = consts.tile([128, CT], F32, tag="tt", name="tt")
        ac = consts.tile([128, CT], F32, tag="ac", name="ac")
        bc = consts.tile([128, CT], F32, tag="bc", name="bc")
        # mean and E[y^2] (gs columns are [s0, q0, s1, q1])
        gsv = gs.rearrange("p (m two) -> p m two", two=2)
        nc.vector.tensor_scalar(
            out=mg[:], in0=gsv[:, :, 0], scalar1=minv, scalar2=None, op0=ALU.mult
        )
        nc.vector.tensor_scalar(
            out=vg[:], in0=gsv[:, :, 1], scalar1=minv, scalar2=None, op0=ALU.mult
        )
        # vg = E[y^2] - mean^2 + eps  (via -(mean^2 - E[y^2]) + eps)
        nc.vector.tensor_tensor(out=tt[:], in0=mg[:], in1=mg[:], op=ALU.mult)
        nc.vector.tensor_tensor(out=vg[:], in0=tt[:], in1=vg[:], op=ALU.subtract)
        nc.vector.tensor_scalar(
            out=vg[:], in0=vg[:], scalar1=-1.0, scalar2=BN_EPS,
            op0=ALU.mult, op1=ALU.add,
        )
        # rstd via quake seed + 3 Newton iterations, all on the DVE
        vg_i = vg.bitcast(mybir.dt.int32)
        rr_i = rr.bitcast(mybir.dt.int32)
        nc.vector.tensor_scalar(
            out=rr_i[:], in0=vg_i[:], scalar1=1, scalar2=None,
            op0=ALU.arith_shift_right,
        )
        nc.vector.tensor_scalar(
            out=rr_i[:], in0=rr_i[:], scalar1=-1, scalar2=0x5F3759DF,
            op0=ALU.mult, op1=ALU.add,
        )
        hv = consts.tile([128, CT], F32, tag="hv", name="hv")
        nc.vector.tensor_scalar(
            out=hv[:], in0=vg[:], scalar1=0.5, scalar2=None, op0=ALU.mult
        )
        for _ in range(3):
            nc.vector.tensor_tensor(out=tt[:], in0=rr[:], in1=rr[:], op=ALU.mult)
            nc.vector.tensor_tensor(out=tt[:], in0=tt[:], in1=hv[:], op=ALU.mult)
            nc.vector.tensor_scalar(
                out=tt[:], in0=tt[:], scalar1=-1.0, scalar2=1.5,
                op0=ALU.mult, op1=ALU.add,
            )
            nc.vector.tensor_tensor(out=rr[:], in0=rr[:], in1=tt[:], op=ALU.mult)
        # A = rstd * gamma ; bc = beta - mean * A
        nc.vector.tensor_tensor(out=ac[:], in0=rr[:], in1=gam_sb[:], op=ALU.mult)
        nc.vector.tensor_tensor(out=bc[:], in0=mg[:], in1=ac[:], op=ALU.mult)
        nc.vector.tensor_tensor(out=bc[:], in0=bet_sb[:], in1=bc[:], op=ALU.subtract)

        # normalize all slices on the DVE (fastest engine for this, and the
        # DMA-issuing queues stay free to pump output transfers); each
        # slice's output DMA is split in two, spread over three queues.
        out_qs = [nc.sync, nc.scalar, nc.gpsimd]
        qi = 0
        for s in range(NSL):
            for mt2 in range(CT):
                sl = slice(s * SL, (s + 1) * SL)
                nc.vector.tensor_scalar(
                    out=y_sb[mt2][:, sl], in0=y_sb[mt2][:, sl],
                    scalar1=ac[:, mt2 : mt2 + 1], scalar2=bc[:, mt2 : mt2 + 1],
                    op0=ALU.mult, op1=ALU.add,
                )
                for sub in range(2):
                    c0 = s * SL + sub * (SL // 2)
                    c1 = c0 + SL // 2
                    out_qs[qi % len(out_qs)].dma_start(
                        out=yout[mt2, :, c0:c1], in_=y_sb[mt2][:, c0:c1]
                    )
                    qi += 1


_NC = None


def _build_nc(debug=False):
    nc = bacc.Bacc(
        "TRN2", target_bir_lowering=False, debug=debug, num_devices=NCORES
    )
    with tile.TileContext(nc, num_cores=NCORES) as tc:
        _emit(tc)
    nc.compile()
    return nc


def _get_nc():
    global _NC
    if _NC is None:
        _NC = _build_nc()
    return _NC


def _prep_in_maps(x, W_filter, b_filter, w_eca, W_proj, gamma, beta):
    bf = ml_dtypes.bfloat16
    x = np.asarray(x, np.float32)
    W_filter = np.asarray(W_filter, np.float32)
    b_filter = np.asarray(b_filter, np.float32)
    w_eca = np.asarray(w_eca, np.float32)
    W_proj = np.asarray(W_proj, np.float32)
    gamma = np.asarray(gamma, np.float32)
    beta = np.asarray(beta, np.float32)

    # zero-padded layout: 66x66 per channel, image at [1:65, 1:65]
    xpad = np.zeros((B, C, XNR, XROW), np.float32)
    xpad[:, :, 1 : 1 + H, 1 : 1 + W] = x
    xp_h = np.ascontiguousarray(
        xpad.reshape(B, CT, 128, XBUF)
    ).astype(bf)

    # permute mm1 weights to TAPORD tap-major: o' = pos*256 + c
    # (original o = c*9 + k)
    wk = W_filter.reshape(C, KS * KS, C).transpose(1, 0, 2)  # [k, o_c, c]
    wperm = wk[TAPORD].reshape(KS * KS * C, C)
    wf_h = np.ascontiguousarray(wperm.T.reshape(CT, 128, MT1 * 128)).astype(bf)
    bk = b_filter.reshape(C, KS * KS).T  # [k, c]
    bperm = bk[TAPORD].reshape(KS * KS * C)
    bfp_h = np.ascontiguousarray(bperm.reshape(MT1, 128).T).astype(np.float32)

    wp_h = np.ascontiguousarray((0.5 * W_proj).T.reshape(CT, 128, C)).astype(bf)
    weca_h = (w_eca / float(HW)).reshape(1, 3).astype(np.float32)
    gam_h = np.ascontiguousarray(gamma.reshape(CT, 128).T).astype(np.float32)
    bet_h = np.ascontiguousarray(beta.reshape(CT, 128).T).astype(np.float32)

    in_maps = []
    for i in range(B):
        m = {
            "wf": wf_h,
            "bfp": bfp_h,
            "wp": wp_h,
            "weca": weca_h,
            "gam": gam_h,
            "bet": bet_h,
            "xp": xp_h[i],
        }
        in_maps.append(m)
    return in_maps


last_result = None


def kernel(x, W_filter, b_filter, w_eca, W_proj, b_proj, gamma, beta):
    """Full-input, full-output DDF module on 8 NeuronCores."""
    global last_result
    # b_proj is mathematically cancelled by the batch-norm; unused.
    in_maps = _prep_in_maps(x, W_filter, b_filter, w_eca, W_proj, gamma, beta)
    nc = _get_nc()
    trace = bool(int(os.environ.get("DDF_TRACE", "0")))
    res = run_bass_kernel_spmd(nc, in_maps, list(range(NCORES)), trace=trace)
    last_result = res
    out = np.stack(
        [res.results[i]["y"].reshape(C, H, W).astype(np.float32) for i in range(B)]
    )
    return out


# revision 45
# speedup vs baseline: 1.1500x; 1.1500x over previous
"""Trainium2 Bass kernel for the DDF (dynamic-filter + ECA + BN) module.

Distribution: data-parallel over batch B=8 across 8 NeuronCores (one image
per core).  All parameters replicated.  BN batch stats are all-reduced
across cores (sync-BN semantics, matching the reference).

Per-core layout: channels on partitions (2 channel-tiles of 128), pixels on
the free dimension.  The per-pixel filter generator (1x1 conv C -> C*9) is
permuted on the host so each PE output m-tile is one (tap, channel-tile)
pair, in the order taps are consumed.

x is shipped ONCE in a 66x66 zero-padded layout (one pad column each side
of every row, one zero guard row top/bottom), so every 3x3 tap window —
including the column-shifted ones — is just an offset strided-AP view of
the same buffer.  This cuts input HBM traffic ~3x vs shipping three
shifted copies (the prologue is HBM-bandwidth-bound with all 8 cores
loading at once) and removes all side-buffer DMA scheduling.

Engine split per chunk: mm1 PSUM evictions (+bias, fp32->bf16) on the
scalar engine, tap products as single paired [128, 2, 16, 64] DVE ops,
the fused-sum add tree on the DVE with the earliest-ready products joining
the mm2 contraction directly, BN stats straight from PSUM.  A dummy
all-reduce at kernel start prepays the ~15 us CC-core collective setup.
"""

import os

import numpy as np
import ml_dtypes

import concourse.bass as bass
import concourse.mybir as mybir
import concourse.tile as tile
from concourse import bacc
from concourse.bass_utils import run_bass_kernel_spmd

B, C, H, W = 8, 256, 64, 64
KS = 3
HW = H * W                    # 4096
XROW = W + 2                  # 66: one zero pad column on each side
XNR = H + 2                   # 66: one zero guard row top and bottom
XBUF = XROW * XNR             # 4356 padded pixels per channel
NCORES = 8
CT = 2                        # channel tiles of 128
MT1 = KS * KS * CT            # 18 mm1 output m-tiles
BN_EPS = 1e-5
F32 = mybir.dt.float32
BF16 = mybir.dt.bfloat16
ROWS_PER_CHUNK = 16
NCHUNKS = H // ROWS_PER_CHUNK  # 4
CHUNK = ROWS_PER_CHUNK * W     # 1024 pixels per chunk per channel-tile
NH = CHUNK // 512              # 512-px matmul groups per chunk

AF = mybir.ActivationFunctionType
ALU = mybir.AluOpType

# Tap order within a chunk: center-window taps (dj==1) first so chunk-0
# compute starts before the side buffers arrive; then left taps, then right.
# All taps go scalar-engine evict (+bias) -> bf16 DVE product: GPSIMD cannot
# read PSUM and its SW elementwise path is ~10x slower than the DVE (and
# poisons DVE throughput via SBUF port contention), so only the scalar and
# vector engines carry the eviction+product work.
TAPORD = [1, 4, 7, 0, 2, 3, 6, 5, 8]


def _emit(tc):
    nc = tc.nc

    # x in padded layout: [ct, c, (row+1)*66 + col + 1], zeros in the pads
    xp = nc.declare_dram_parameter("xp", [CT, 128, XBUF], BF16, isOutput=False)
    wf = nc.declare_dram_parameter("wf", [CT, 128, MT1 * 128], BF16, isOutput=False)
    bfp = nc.declare_dram_parameter("bfp", [128, MT1], F32, isOutput=False)
    wp = nc.declare_dram_parameter("wp", [CT, 128, C], BF16, isOutput=False)
    weca = nc.declare_dram_parameter("weca", [1, 3], F32, isOutput=False)
    gam = nc.declare_dram_parameter("gam", [128, CT], F32, isOutput=False)
    bet = nc.declare_dram_parameter("bet", [128, CT], F32, isOutput=False)
    yout = nc.declare_dram_parameter("y", [CT, 128, HW], F32, isOutput=True)

    with (
        tc.tile_pool(name="consts", bufs=1) as consts,
        tc.tile_pool(name="fps", bufs=3, space="PSUM") as fps,
        tc.tile_pool(name="yps", bufs=2, space="PSUM") as yps,
        tc.tile_pool(name="fsb", bufs=4) as fsb_pool,
        tc.tile_pool(name="prod", bufs=2) as prod_pool,
        tc.tile_pool(name="dram", bufs=1, space="DRAM") as dram,
    ):
        # ---- resident tensors -------------------------------------------
        wf_sb = [consts.tile([128, MT1 * 128], BF16, tag=f"wf{kt}", name=f"wf{kt}")
                 for kt in range(CT)]
        wp_sb = [consts.tile([128, C], BF16, tag=f"wp{kt}", name=f"wp{kt}")
                 for kt in range(CT)]
        bfp_sb = consts.tile([128, MT1], F32, tag="bfp", name="bfp")
        gam_sb = consts.tile([128, CT], F32, tag="gam", name="gam")
        bet_sb = consts.tile([128, CT], F32, tag="bet", name="bet")
        wecab = consts.tile([128, 3], F32, tag="wecab", name="wecab")
        # both channel-tiles of padded x in one tile, so a single strided-AP
        # DVE op can process both ct halves of a tap
        xp_sb = consts.tile([128, CT * XBUF], BF16, tag="xp", name="xp")
        y_sb = [consts.tile([128, HW], F32, tag=f"ysb{mt}", name=f"ysb{mt}")
                for mt in range(CT)]
        stats_sb = [
            consts.tile([128, NCHUNKS * NH, 6], F32, tag=f"st{mt}", name=f"st{mt}")
            for mt in range(CT)
        ]

        # ---- input DMA ---------------------------------------------------
        # All input flows in strict need order on two queues.  The prologue
        # is HBM-bandwidth bound (8 cores load concurrently; completions are
        # roughly fair-shared), so nothing non-critical may run early.
        def xp_dma(q, ct, r0, r1):
            """Padded rows r0..r1 (buffer row index, 0..66) of channel-tile ct."""
            q.dma_start(
                out=xp_sb[:, ct * XBUF + r0 * XROW : ct * XBUF + r1 * XROW],
                in_=xp[ct, :, r0 * XROW : r1 * XROW],
            )

        # wf is laid out in TAPORD order: slice A = tap positions 0-1,
        # B = 2-4, C = 5-6, D = 7-8 (columns of 128 per (pos, ct) tile).
        def wf_dma(q, kt, c0, c1):
            q.dma_start(out=wf_sb[kt][:, c0:c1], in_=wf[kt, :, c0:c1])

        # Everything flows on ONE queue in strict need order: the queue's
        # DMA ring (~4-5 in flight) completes roughly FIFO, so this is the
        # only way to guarantee the critical pieces finish first when all
        # 8 cores share HBM bandwidth.
        sp = nc.sync
        for kt in range(CT):
            wf_dma(sp, kt, 0, 512)              # slice A (pos 0-1)
        for ct in range(CT):
            xp_dma(sp, ct, 0, 10)               # image rows -1..8
        for ct in range(CT):
            xp_dma(sp, ct, 10, 18)              # image rows 9..16
        sp.dma_start(out=bfp_sb[:], in_=bfp[:, :])
        for kt in range(CT):
            wf_dma(sp, kt, 512, 1280)           # slice B (pos 2-4)
        for kt in range(CT):
            wf_dma(sp, kt, 1280, 1792)          # slice C (pos 5-6)
        for ct in range(CT):
            xp_dma(sp, ct, 18, 34)              # chunk 1
        for kt in range(CT):
            wf_dma(sp, kt, 1792, MT1 * 128)     # slice D (pos 7-8)
        for kt in range(CT):
            sp.dma_start(out=wp_sb[kt][:], in_=wp[kt])
        for ct in range(CT):
            xp_dma(sp, ct, 34, 50)              # chunk 2
        sp.dma_start(out=wecab[:], in_=weca[0:1, :].to_broadcast([128, 3]))
        for ct in range(CT):
            xp_dma(sp, ct, 50, XNR)             # chunk 3 (+ bottom guard)
        sp.dma_start(out=gam_sb[:], in_=gam[:, :])
        sp.dma_start(out=bet_sb[:], in_=bet[:, :])

        xpv = xp_sb.rearrange("p (t r c) -> p t r c", t=CT, c=XROW)

        def win_mm(kt, row0, nrows):
            """Center window rows row0..row0+nrows as a [128, nrows, 64]
            strided AP for the mm1 rhs (channel-tile kt)."""
            return xpv[:, kt, row0 + 1 : row0 + 1 + nrows, 1 : 1 + W]

        def win_ct(ct, row0, dj=1, nrows=ROWS_PER_CHUNK):
            """One channel-tile of a (dj-shifted) window, [128, nrows, 64]."""
            return xpv[:, ct, row0 + 1 : row0 + 1 + nrows, dj : dj + W]

        def win2(dj, row0, nrows=ROWS_PER_CHUNK):
            """Both channel-tiles of a dj-shifted window as one
            [128, 2, nrows, 64] AP (pads supply the shifted-in zeros)."""
            return xpv[:, :, row0 + 1 : row0 + 1 + nrows, dj : dj + W]

        # ---- warmup collective ------------------------------------------
        # The CC sidecar core takes ~15 us of setup between the trigger and
        # the start of the mesh algorithm.  Fire a dummy all-reduce at kernel
        # start so that setup (ring/channel init) overlaps the main loop
        # instead of sitting on the critical path of the BN-stats reduce.
        wrm = consts.tile([128, 1], F32, tag="wrm", name="wrm")
        nc.vector.memset(wrm[:], 0.0)
        wrm_in = dram.tile([128, 1], F32, tag="wrmi", name="wrmi")
        wrm_out = dram.tile([128, 1], F32, tag="wrmo", name="wrmo")
        nc.gpsimd.dma_start(out=wrm_in[:], in_=wrm[:])
        nc.gpsimd.collective_compute(
            "AllReduce",
            ALU.add,
            replica_groups=[list(range(NCORES))],
            ins=[wrm_in[:].opt()],
            outs=[wrm_out[:].opt()],
        )

        # ---- ECA state tiles (filled inside the chunk loop so no engine
        # queue stalls waiting for the full-image pooled sum) --------------
        poolp = consts.tile([128, CT, NCHUNKS], F32, tag="poolp", name="poolp")
        pool2 = consts.tile([128, CT], F32, tag="pool2", name="pool2")
        shd = consts.tile([128, CT], F32, tag="shd", name="shd")  # pooled[c-1]
        shu = consts.tile([128, CT], F32, tag="shu", name="shu")  # pooled[c+1]
        eca1 = consts.tile([128, CT], F32, tag="eca1", name="eca1")
        eca2 = consts.tile([128, CT], F32, tag="eca2", name="eca2")
        attn = consts.tile([128, CT], F32, tag="attn", name="attn")
        nc.vector.memset(shd[:], 0.0)
        nc.vector.memset(shu[:], 0.0)

        xpf = xp_sb.rearrange("p (t x) -> p t x", t=CT)

        def pool_reduce(ci):
            # flat padded span of the chunk's rows; the pad zeros are
            # harmless in the sum
            a = (1 + ROWS_PER_CHUNK * ci) * XROW
            b = a + ROWS_PER_CHUNK * XROW
            nc.vector.tensor_reduce(
                out=poolp[:, :, ci : ci + 1],
                in_=xpf[:, :, a:b],
                axis=mybir.AxisListType.X,
                op=ALU.add,
            )

        def eca_attn():
            """pooled -> attn. Channel shifts cross the two channel-tiles
            via tiny partition-offset DMAs (on the sync queue, which is done
            with its input DMAs by now)."""
            for ct in range(CT):
                nc.vector.tensor_reduce(
                    out=pool2[:, ct : ct + 1],
                    in_=poolp[:, ct, :],
                    axis=mybir.AxisListType.X,
                    op=ALU.add,
                )
            for ct in range(CT):
                nc.sync.dma_start(
                    out=shd[1:128, ct : ct + 1], in_=pool2[0:127, ct : ct + 1]
                )
                nc.sync.dma_start(
                    out=shu[0:127, ct : ct + 1], in_=pool2[1:128, ct : ct + 1]
                )
            nc.sync.dma_start(out=shd[0:1, 1:2], in_=pool2[127:128, 0:1])
            nc.sync.dma_start(out=shu[127:128, 0:1], in_=pool2[0:1, 1:2])
            nc.vector.tensor_scalar(
                out=eca1, in0=shd[:], scalar1=wecab[:, 0:1], scalar2=None,
                op0=ALU.mult,
            )
            nc.vector.scalar_tensor_tensor(
                out=eca2, in0=pool2[:], scalar=wecab[:, 1:2], in1=eca1[:],
                op0=ALU.mult, op1=ALU.add,
            )
            nc.vector.scalar_tensor_tensor(
                out=eca1, in0=shu[:], scalar=wecab[:, 2:3], in1=eca2[:],
                op0=ALU.mult, op1=ALU.add,
            )
            # attn = sigmoid(eca) = 1 / (1 + exp(-eca))
            nc.scalar.activation(out=eca2[:], in_=eca1[:], func=AF.Exp, scale=-1.0)
            nc.vector.tensor_scalar(
                out=attn, in0=eca2[:], scalar1=1.0, scalar2=None, op0=ALU.add
            )
            nc.vector.reciprocal(out=attn[:], in_=attn[:])

        def make_cfb(r0):
            cfb = prod_pool.tile([128, CT * CHUNK], BF16, tag="cf", name="cf")
            for ct in range(CT):
                dst = cfb[:, ct * CHUNK : (ct + 1) * CHUNK]
                nc.scalar.activation(
                    out=dst.rearrange("p (r c) -> p r c", c=W),
                    in_=win_ct(ct, r0), func=AF.Identity,
                    scale=attn[:, ct : ct + 1],
                )
            return cfb

        # ---- main loop over row chunks ----------------------------------
        for ci in range(NCHUNKS):
            r0 = ci * ROWS_PER_CHUNK
            prods = {}
            cfb = None
            for pos, k in enumerate(TAPORD):
                if ci == 0 and pos == 1:
                    pool_reduce(0)
                di, dj = divmod(k, KS)
                pr = prod_pool.tile([128, CT * CHUNK], BF16, tag=f"pr{k}",
                                    name=f"pr{k}")
                fsb = fsb_pool.tile([128, CT * CHUNK], BF16, tag="fsb",
                                    name="fsb")
                for ct in range(CT):
                    mt = pos * CT + ct
                    fp = fps.tile([128, CHUNK], F32, tag="fp", name="fp")
                    for nh in range(NH):
                        for kt in range(CT):
                            lhsT = wf_sb[kt][:, mt * 128 : (mt + 1) * 128]
                            rhs = win_mm(kt, r0 + nh * 8, 8)
                            nc.tensor.matmul(
                                fp[:, nh * 512 : (nh + 1) * 512],
                                lhsT,
                                rhs,
                                start=(kt == 0),
                                stop=(kt == CT - 1),
                            )
                    # scalar-engine evict (+bias, fp32->bf16)
                    nc.scalar.activation(
                        out=fsb[:, ct * CHUNK : (ct + 1) * CHUNK], in_=fp[:],
                        func=AF.Identity, bias=bfp_sb[:, mt : mt + 1],
                        scale=1.0,
                    )
                # one DVE product covers both channel-tiles via strided APs
                nc.vector.tensor_tensor(
                    out=pr.rearrange("p (t r c) -> p t r c", t=CT, c=W),
                    in0=fsb.rearrange("p (t r c) -> p t r c", t=CT, c=W),
                    in1=win2(dj, r0 + di - 1),
                    op=ALU.mult,
                )
                prods[k] = pr

                # interleave adds / attention work as results become available;
                # the tree is arranged so only ONE add remains after the last
                # tap's product (short chunk tail).
                if pos == 2:
                    if ci > 0:
                        cfb = make_cfb(r0)
                elif pos == 3:
                    if ci > 0:
                        nc.vector.tensor_add(prods[0][:], prods[0][:], cfb[:])
                    else:
                        pool_reduce(1)
                elif pos == 4:
                    nc.vector.tensor_add(prods[0][:], prods[0][:], prods[2][:])
                elif pos == 5:
                    if ci != NCHUNKS - 1:
                        nc.vector.tensor_add(prods[4][:], prods[4][:], prods[0][:])
                    if ci == 0:
                        pool_reduce(2)
                elif pos == 6:
                    nc.vector.tensor_add(prods[3][:], prods[3][:], prods[6][:])
                elif pos == 7:
                    if ci == 0:
                        # all center pieces have arrived by now; finish the
                        # pooled sum and the attention on otherwise-idle slack
                        pool_reduce(3)
                        eca_attn()
                        cfb = make_cfb(r0)
                    nc.vector.tensor_add(prods[3][:], prods[3][:], prods[5][:])
                    if ci != NCHUNKS - 1:
                        nc.vector.tensor_add(prods[4][:], prods[4][:], prods[3][:])
                elif pos == 8:
                    nc.vector.tensor_add(prods[4][:], prods[4][:], prods[8][:])

            # projection matmul + y eviction + local BN stats.
            # fused tree result lives in prods[4]; p7 and p1 (ready earliest)
            # join the contraction directly, trading PE cycles for DVE adds.
            # In chunk 0 the attention term joins the contraction too (it is
            # only ready late there).
            mm2_srcs = [prods[7], prods[1], prods[4]]
            if ci == 0:
                mm2_srcs.append(cfb)
            elif ci == NCHUNKS - 1:
                # last chunk: keep the DVE tail short by letting the PE
                # contract the p0/p3 sub-roots directly (it idles here)
                mm2_srcs = [prods[7], prods[1], prods[0], prods[3], prods[4]]
            for mt2 in range(CT):
                ypt = [yps.tile([128, 512], F32, tag="yp", name="yp")
                       for _ in range(NH)]
                nsrc = len(mm2_srcs)
                for si, srct in enumerate(mm2_srcs):
                    for kt in range(CT):
                        lhsT2 = wp_sb[kt][:, mt2 * 128 : (mt2 + 1) * 128]
                        for nh in range(NH):
                            nc.tensor.matmul(
                                ypt[nh][:],
                                lhsT2,
                                srct[:, kt * CHUNK + nh * 512 : kt * CHUNK + (nh + 1) * 512],
                                start=(si == 0 and kt == 0),
                                stop=(si == nsrc - 1 and kt == CT - 1),
                            )
                for nh in range(NH):
                    dst = y_sb[mt2][:, r0 * W + nh * 512 : r0 * W + (nh + 1) * 512]
                    # stats straight from PSUM, in parallel with the evict
                    nc.vector.bn_stats(
                        out=stats_sb[mt2][:, ci * NH + nh, :], in_=ypt[nh][:]
                    )
                    nc.scalar.activation(out=dst, in_=ypt[nh][:], func=AF.Copy)

        # ---- global BN stats via all-reduce -----------------------------
        ps = consts.tile([128, 2 * CT], F32, tag="ps", name="ps")
        for mt2 in range(CT):
            mv = consts.tile([128, 2], F32, tag=f"mv{mt2}", name=f"mv{mt2}")
            nc.vector.bn_aggr(out=mv[:], in_=stats_sb[mt2][:])
            mean = mv[:, 0:1]
            var = mv[:, 1:2]
            nc.vector.tensor_scalar(
                out=ps[:, 2 * mt2 : 2 * mt2 + 1], in0=mean, scalar1=float(HW),
                scalar2=None, op0=ALU.mult,
            )
            # sumsq = (var + mean^2) * HW
            nc.vector.scalar_tensor_tensor(
                out=ps[:, 2 * mt2 + 1 : 2 * mt2 + 2], in0=mean, scalar=mean,
                in1=var, op0=ALU.mult, op1=ALU.add,
            )
            nc.vector.tensor_scalar(
                out=ps[:, 2 * mt2 + 1 : 2 * mt2 + 2],
                in0=ps[:, 2 * mt2 + 1 : 2 * mt2 + 2],
                scalar1=float(HW), scalar2=None, op0=ALU.mult,
            )

        ps_b = dram.tile([128, 2 * CT], F32, tag="psb", name="psb")
        gs_b = dram.tile([128, 2 * CT], F32, tag="gsb", name="gsb")
        nc.sync.dma_start(out=ps_b[:], in_=ps[:])
        nc.gpsimd.collective_compute(
            "AllReduce",
            ALU.add,
            replica_groups=[list(range(NCORES))],
            ins=[ps_b[:].opt()],
            outs=[gs_b[:].opt()],
        )
        gs = consts.tile([128, 2 * CT], F32, tag="gs", name="gs")
        nc.scalar.dma_start(out=gs[:], in_=gs_b[:])

        # ---- normalize and write out ------------------------------------
        minv = 1.0 / float(B * HW)
        NSL = 4  # normalize/store slices per channel-tile
        SL = HW // NSL
        mg = consts.tile([128, CT], F32, tag="mg", name="mg")
        vg = consts.tile([128, CT], F32, tag="vg", name="vg")
        rr = consts.tile([128, CT], F32, tag="rr", name="rr")
        tt = consts.tile([128, CT], F32, tag="tt", name="tt")
        ac = consts.tile([128, CT], F32, tag="ac", name="ac")
        bc = consts.tile([128, CT], F32, tag="bc", name="bc")
        # mean and E[y^2] (gs columns are [s0, q0, s1, q1])
        gsv = gs.rearrange("p (m two) -> p m two", two=2)
        nc.vector.tensor_scalar(
            out=mg[:], in0=gsv[:, :, 0], scalar1=minv, scalar2=None, op0=ALU.mult
        )
        nc.vector.tensor_scalar(
            out=vg[:], in0=gsv[:, :, 1], scalar1=minv, scalar2=None, op0=ALU.mult
        )
        # vg = E[y^2] - mean^2 + eps  (via -(mean^2 - E[y^2]) + eps)
        nc.vector.tensor_tensor(out=tt[:], in0=mg[:], in1=mg[:], op=ALU.mult)
        nc.vector.tensor_tensor(out=vg[:], in0=tt[:], in1=vg[:], op=ALU.subtract)
        nc.vector.tensor_scalar(
            out=vg[:], in0=vg[:], scalar1=-1.0, scalar2=BN_EPS,
            op0=ALU.mult, op1=ALU.add,
        )
        # rstd via quake seed + 3 Newton iterations, all on the DVE
        vg_i = vg.bitcast(mybir.dt.int32)
        rr_i = rr.bitcast(mybir.dt.int32)
        nc.vector.tensor_scalar(
            out=rr_i[:], in0=vg_i[:], scalar1=1, scalar2=None,
            op0=ALU.arith_shift_right,
        )
        nc.vector.tensor_scalar(
            out=rr_i[:], in0=rr_i[:], scalar1=-1, scalar2=0x5F3759DF,
            op0=ALU.mult, op1=ALU.add,
        )
        hv = consts.tile([128, CT], F32, tag="hv", name="hv")
        nc.vector.tensor_scalar(
            out=hv[:], in0=vg[:], scalar1=0.5, scalar2=None, op0=ALU.mult
        )
        for _ in range(3):
            nc.vector.tensor_tensor(out=tt[:], in0=rr[:], in1=rr[:], op=ALU.mult)
            nc.vector.tensor_tensor(out=tt[:], in0=tt[:], in1=hv[:], op=ALU.mult)
            nc.vector.tensor_scalar(
                out=tt[:], in0=tt[:], scalar1=-1.0, scalar2=1.5,
                op0=ALU.mult, op1=ALU.add,
            )
            nc.vector.tensor_tensor(out=rr[:], in0=rr[:], in1=tt[:], op=ALU.mult)
        # A = rstd * gamma ; bc = beta - mean * A
        nc.vector.tensor_tensor(out=ac[:], in0=rr[:], in1=gam_sb[:], op=ALU.mult)
        nc.vector.tensor_tensor(out=bc[:], in0=mg[:], in1=ac[:], op=ALU.mult)
        nc.vector.tensor_tensor(out=bc[:], in0=bet_sb[:], in1=bc[:], op=ALU.subtract)

        # normalize all slices on the DVE (fastest engine for this, and the
        # DMA-issuing queues stay free to pump output transfers); each
        # slice's output DMA is split in two, spread over three queues.
        out_qs = [nc.sync, nc.scalar, nc.gpsimd]
        qi = 0
        for s in range(NSL):
            for mt2 in range(CT):
                sl = slice(s * SL, (s + 1) * SL)
                nc.vector.tensor_scalar(
                    out=y_sb[mt2][:, sl], in0=y_sb[mt2][:, sl],
                    scalar1=ac[:, mt2 : mt2 + 1], scalar2=bc[:, mt2 : mt2 + 1],
                    op0=ALU.mult, op1=ALU.add,
                )
                for sub in range(2):
                    c0 = s * SL + sub * (SL // 2)
                    c1 = c0 + SL // 2
                    out_qs[qi % len(out_qs)].dma_start(
                        out=yout[mt2, :, c0:c1], in_=y_sb[mt2][:, c0:c1]
                    )
                    qi += 1


_NC = None


def _build_nc(debug=False):
    nc = bacc.Bacc(
        "TRN2", target_bir_lowering=False, debug=debug, num_devices=NCORES
    )
    with tile.TileContext(nc, num_cores=NCORES) as tc:
        _emit(tc)
    nc.compile()
    return nc


def _get_nc():
    global _NC
    if _NC is None:
        _NC = _build_nc()
    return _NC


def _prep_in_maps(x, W_filter, b_filter, w_eca, W_proj, gamma, beta):
    bf = ml_dtypes.bfloat16
    x = np.asarray(x, np.float32)
    W_filter = np.asarray(W_filter, np.float32)
    b_filter = np.asarray(b_filter, np.float32)
    w_eca = np.asarray(w_eca, np.float32)
    W_proj = np.asarray(W_proj, np.float32)
    gamma = np.asarray(gamma, np.float32)
    beta = np.asarray(beta, np.float32)

    # zero-padded layout: 66x66 per channel, image at [1:65, 1:65]
    xpad = np.zeros((B, C, XNR, XROW), np.float32)
    xpad[:, :, 1 : 1 + H, 1 : 1 + W] = x
    xp_h = np.ascontiguousarray(
        xpad.reshape(B, CT, 128, XBUF)
    ).astype(bf)

    # permute mm1 weights to TAPORD tap-major: o' = pos*256 + c
    # (original o = c*9 + k)
    wk = W_filter.reshape(C, KS * KS, C).transpose(1, 0, 2)  # [k, o_c, c]
    wperm = wk[TAPORD].reshape(KS * KS * C, C)
    wf_h = np.ascontiguousarray(wperm.T.reshape(CT, 128, MT1 * 128)).astype(bf)
    bk = b_filter.reshape(C, KS * KS).T  # [k, c]
    bperm = bk[TAPORD].reshape(KS * KS * C)
    bfp_h = np.ascontiguousarray(bperm.reshape(MT1, 128).T).astype(np.float32)

    wp_h = np.ascontiguousarray((0.5 * W_proj).T.reshape(CT, 128, C)).astype(bf)
    weca_h = (w_eca / float(HW)).reshape(1, 3).astype(np.float32)
    gam_h = np.ascontiguousarray(gamma.reshape(CT, 128).T).astype(np.float32)
    bet_h = np.ascontiguousarray(beta.reshape(CT, 128).T).astype(np.float32)

    in_maps = []
    for i in range(B):
        m = {
            "wf": wf_h,
            "bfp": bfp_h,
            "wp": wp_h,
            "weca": weca_h,
            "gam": gam_h,
            "bet": bet_h,
            "xp": xp_h[i],
        }
        in_maps.append(m)
    return in_maps


last_result = None


def kernel(x, W_filter, b_filter, w_eca, W_proj, b_proj, gamma, beta):
    """Full-input, full-output DDF module on 8 NeuronCores."""
    global last_result
    # b_proj is mathematically cancelled by the batch-norm; unused.
    in_maps = _prep_in_maps(x, W_filter, b_filter, w_eca, W_proj, gamma, beta)
    nc = _get_nc()
    trace = bool(int(os.environ.get("DDF_TRACE", "0")))
    res = run_bass_kernel_spmd(nc, in_maps, list(range(NCORES)), trace=trace)
    last_result = res
    out = np.stack(
        [res.results[i]["y"].reshape(C, H, W).astype(np.float32) for i in range(B)]
    )
    return out


# revision 54
# speedup vs baseline: 1.2472x; 1.0844x over previous
"""Trainium2 Bass kernel for the DDF (dynamic-filter + ECA + BN) module.

Distribution: data-parallel over batch B=8 across 8 NeuronCores (one image
per core).  All parameters replicated.  BN batch stats are all-reduced
across cores (sync-BN semantics, matching the reference).

Per-core layout: channels on partitions (2 channel-tiles of 128), pixels on
the free dimension.  The per-pixel filter generator (1x1 conv C -> C*9) is
permuted on the host so each PE output m-tile is one (tap, channel-tile)
pair, in the order taps are consumed.

x is shipped ONCE in a 66x66 zero-padded layout (one pad column each side
of every row, one zero guard row top/bottom), so every 3x3 tap window —
including the column-shifted ones — is just an offset strided-AP view of
the same buffer.  This cuts input HBM traffic ~3x vs shipping three
shifted copies (the prologue is HBM-bandwidth-bound with all 8 cores
loading at once) and removes all side-buffer DMA scheduling.

Engine split per chunk: mm1 PSUM evictions (+bias, fp32->bf16) on the
scalar engine, tap products as single paired [128, 2, 16, 64] DVE ops,
the fused-sum add tree on the DVE with the earliest-ready products joining
the mm2 contraction directly, BN stats straight from PSUM.  A dummy
all-reduce at kernel start prepays the ~15 us CC-core collective setup.
"""

import os

import numpy as np
import ml_dtypes

import concourse.bass as bass
import concourse.mybir as mybir
import concourse.tile as tile
from concourse import bacc
from concourse.bass_utils import run_bass_kernel_spmd

B, C, H, W = 8, 256, 64, 64
KS = 3
HW = H * W                    # 4096
XROW = W + 2                  # 66: one zero pad column on each side
XNR = H + 2                   # 66: one zero guard row top and bottom
XBUF = XROW * XNR             # 4356 padded pixels per channel
NCORES = 8
CT = 2                        # channel tiles of 128
MT1 = KS * KS * CT            # 18 mm1 output m-tiles
BN_EPS = 1e-5
F32 = mybir.dt.float32
BF16 = mybir.dt.bfloat16
ROWS_PER_CHUNK = 16
NCHUNKS = H // ROWS_PER_CHUNK  # 4
CHUNK = ROWS_PER_CHUNK * W     # 1024 pixels per chunk per channel-tile
NH = CHUNK // 512              # 512-px matmul groups per chunk

AF = mybir.ActivationFunctionType
ALU = mybir.AluOpType

# Tap order within a chunk: center-window taps (dj==1) first so chunk-0
# compute starts before the side buffers arrive; then left taps, then right.
# All taps go scalar-engine evict (+bias) -> bf16 DVE product: GPSIMD cannot
# read PSUM and its SW elementwise path is ~10x slower than the DVE (and
# poisons DVE throughput via SBUF port contention), so only the scalar and
# vector engines carry the eviction+product work.
TAPORD = [1, 4, 7, 0, 2, 3, 6, 5, 8]


def _emit(tc):
    nc = tc.nc

    # x in padded layout: [ct, c, (row+1)*66 + col + 1], zeros in the pads
    xp = nc.declare_dram_parameter("xp", [CT, 128, XBUF], BF16, isOutput=False)
    wf = nc.declare_dram_parameter("wf", [CT, 128, MT1 * 128], BF16, isOutput=False)
    bfp = nc.declare_dram_parameter("bfp", [128, MT1], F32, isOutput=False)
    wp = nc.declare_dram_parameter("wp", [CT, 128, C], BF16, isOutput=False)
    weca = nc.declare_dram_parameter("weca", [1, 3], F32, isOutput=False)
    gam = nc.declare_dram_parameter("gam", [128, CT], F32, isOutput=False)
    bet = nc.declare_dram_parameter("bet", [128, CT], F32, isOutput=False)
    yout = nc.declare_dram_parameter("y", [CT, 128, HW], F32, isOutput=True)

    with (
        tc.tile_pool(name="consts", bufs=1) as consts,
        tc.tile_pool(name="fps", bufs=3, space="PSUM") as fps,
        tc.tile_pool(name="yps", bufs=2, space="PSUM") as yps,
        tc.tile_pool(name="fsb", bufs=4) as fsb_pool,
        tc.tile_pool(name="prod", bufs=2) as prod_pool,
        tc.tile_pool(name="dram", bufs=1, space="DRAM") as dram,
    ):
        # ---- resident tensors -------------------------------------------
        wf_sb = [consts.tile([128, MT1 * 128], BF16, tag=f"wf{kt}", name=f"wf{kt}")
                 for kt in range(CT)]
        wp_sb = [consts.tile([128, C], BF16, tag=f"wp{kt}", name=f"wp{kt}")
                 for kt in range(CT)]
        bfp_sb = consts.tile([128, MT1], F32, tag="bfp", name="bfp")
        gam_sb = consts.tile([128, CT], F32, tag="gam", name="gam")
        bet_sb = consts.tile([128, CT], F32, tag="bet", name="bet")
        wecab = consts.tile([128, 3], F32, tag="wecab", name="wecab")
        # both channel-tiles of padded x in one tile, so a single strided-AP
        # DVE op can process both ct halves of a tap
        xp_sb = consts.tile([128, CT * XBUF], BF16, tag="xp", name="xp")
        y_sb = [consts.tile([128, HW], F32, tag=f"ysb{mt}", name=f"ysb{mt}")
                for mt in range(CT)]
        stats_sb = [
            consts.tile([128, NCHUNKS * NH, 6], F32, tag=f"st{mt}", name=f"st{mt}")
            for mt in range(CT)
        ]

        # ---- input DMA ---------------------------------------------------
        # All input flows in strict need order on two queues.  The prologue
        # is HBM-bandwidth bound (8 cores load concurrently; completions are
        # roughly fair-shared), so nothing non-critical may run early.
        def xp_dma(q, ct, r0, r1):
            """Padded rows r0..r1 (buffer row index, 0..66) of channel-tile ct."""
            q.dma_start(
                out=xp_sb[:, ct * XBUF + r0 * XROW : ct * XBUF + r1 * XROW],
                in_=xp[ct, :, r0 * XROW : r1 * XROW],
            )

        # wf is laid out in TAPORD order: slice A = tap positions 0-1,
        # B = 2-4, C = 5-6, D = 7-8 (columns of 128 per (pos, ct) tile).
        def wf_dma(q, kt, c0, c1):
            q.dma_start(out=wf_sb[kt][:, c0:c1], in_=wf[kt, :, c0:c1])

        # Everything flows on ONE queue in strict need order: the queue's
        # DMA ring (~4-5 in flight) completes roughly FIFO, so this is the
        # only way to guarantee the critical pieces finish first when all
        # 8 cores share HBM bandwidth.
        sp = nc.sync
        for kt in range(CT):
            wf_dma(sp, kt, 0, 512)              # slice A (pos 0-1)
        for ct in range(CT):
            xp_dma(sp, ct, 0, 10)               # image rows -1..8
        for ct in range(CT):
            xp_dma(sp, ct, 10, 18)              # image rows 9..16
        sp.dma_start(out=bfp_sb[:], in_=bfp[:, :])
        for kt in range(CT):
            wf_dma(sp, kt, 512, 1280)           # slice B (pos 2-4)
        for kt in range(CT):
            wf_dma(sp, kt, 1280, 1792)          # slice C (pos 5-6)
        for ct in range(CT):
            xp_dma(sp, ct, 18, 34)              # chunk 1
        for kt in range(CT):
            wf_dma(sp, kt, 1792, MT1 * 128)     # slice D (pos 7-8)
        for kt in range(CT):
            sp.dma_start(out=wp_sb[kt][:], in_=wp[kt])
        for ct in range(CT):
            xp_dma(sp, ct, 34, 50)              # chunk 2
        sp.dma_start(out=wecab[:], in_=weca[0:1, :].to_broadcast([128, 3]))
        for ct in range(CT):
            xp_dma(sp, ct, 50, XNR)             # chunk 3 (+ bottom guard)
        sp.dma_start(out=gam_sb[:], in_=gam[:, :])
        sp.dma_start(out=bet_sb[:], in_=bet[:, :])

        xpv = xp_sb.rearrange("p (t r c) -> p t r c", t=CT, c=XROW)

        def win_mm(kt, row0, nrows):
            """Center window rows row0..row0+nrows as a [128, nrows, 64]
            strided AP for the mm1 rhs (channel-tile kt)."""
            return xpv[:, kt, row0 + 1 : row0 + 1 + nrows, 1 : 1 + W]

        def win_ct(ct, row0, dj=1, nrows=ROWS_PER_CHUNK):
            """One channel-tile of a (dj-shifted) window, [128, nrows, 64]."""
            return xpv[:, ct, row0 + 1 : row0 + 1 + nrows, dj : dj + W]

        def win2(dj, row0, nrows=ROWS_PER_CHUNK):
            """Both channel-tiles of a dj-shifted window as one
            [128, 2, nrows, 64] AP (pads supply the shifted-in zeros)."""
            return xpv[:, :, row0 + 1 : row0 + 1 + nrows, dj : dj + W]

        # ---- warmup collective ------------------------------------------
        # The CC sidecar core takes ~15 us of setup between the trigger and
        # the start of the mesh algorithm.  Fire a dummy all-reduce at kernel
        # start so that setup (ring/channel init) overlaps the main loop
        # instead of sitting on the critical path of the BN-stats reduce.
        wrm = consts.tile([128, 1], F32, tag="wrm", name="wrm")
        nc.vector.memset(wrm[:], 0.0)
        wrm_in = dram.tile([128, 1], F32, tag="wrmi", name="wrmi")
        wrm_out = dram.tile([128, NCORES], F32, tag="wrmo", name="wrmo")
        nc.gpsimd.dma_start(out=wrm_in[:], in_=wrm[:])
        nc.gpsimd.collective_compute(
            "AllGather",
            ALU.bypass,
            replica_groups=[list(range(NCORES))],
            ins=[wrm_in[:].opt()],
            outs=[wrm_out[:].opt()],
        )

        # ---- ECA state tiles (filled inside the chunk loop so no engine
        # queue stalls waiting for the full-image pooled sum) --------------
        poolp = consts.tile([128, CT, NCHUNKS], F32, tag="poolp", name="poolp")
        pool2 = consts.tile([128, CT], F32, tag="pool2", name="pool2")
        shd = consts.tile([128, CT], F32, tag="shd", name="shd")  # pooled[c-1]
        shu = consts.tile([128, CT], F32, tag="shu", name="shu")  # pooled[c+1]
        eca1 = consts.tile([128, CT], F32, tag="eca1", name="eca1")
        eca2 = consts.tile([128, CT], F32, tag="eca2", name="eca2")
        attn = consts.tile([128, CT], F32, tag="attn", name="attn")
        nc.vector.memset(shd[:], 0.0)
        nc.vector.memset(shu[:], 0.0)

        xpf = xp_sb.rearrange("p (t x) -> p t x", t=CT)

        def pool_reduce(ci):
            # flat padded span of the chunk's rows; the pad zeros are
            # harmless in the sum
            a = (1 + ROWS_PER_CHUNK * ci) * XROW
            b = a + ROWS_PER_CHUNK * XROW
            nc.vector.tensor_reduce(
                out=poolp[:, :, ci : ci + 1],
                in_=xpf[:, :, a:b],
                axis=mybir.AxisListType.X,
                op=ALU.add,
            )

        def eca_attn():
            """pooled -> attn. Channel shifts cross the two channel-tiles
            via tiny partition-offset DMAs (on the sync queue, which is done
            with its input DMAs by now)."""
            for ct in range(CT):
                nc.vector.tensor_reduce(
                    out=pool2[:, ct : ct + 1],
                    in_=poolp[:, ct, :],
                    axis=mybir.AxisListType.X,
                    op=ALU.add,
                )
            for ct in range(CT):
                nc.sync.dma_start(
                    out=shd[1:128, ct : ct + 1], in_=pool2[0:127, ct : ct + 1]
                )
                nc.sync.dma_start(
                    out=shu[0:127, ct : ct + 1], in_=pool2[1:128, ct : ct + 1]
                )
            nc.sync.dma_start(out=shd[0:1, 1:2], in_=pool2[127:128, 0:1])
            nc.sync.dma_start(out=shu[127:128, 0:1], in_=pool2[0:1, 1:2])
            nc.vector.tensor_scalar(
                out=eca1, in0=shd[:], scalar1=wecab[:, 0:1], scalar2=None,
                op0=ALU.mult,
            )
            nc.vector.scalar_tensor_tensor(
                out=eca2, in0=pool2[:], scalar=wecab[:, 1:2], in1=eca1[:],
                op0=ALU.mult, op1=ALU.add,
            )
            nc.vector.scalar_tensor_tensor(
                out=eca1, in0=shu[:], scalar=wecab[:, 2:3], in1=eca2[:],
                op0=ALU.mult, op1=ALU.add,
            )
            # attn = sigmoid(eca) = 1 / (1 + exp(-eca))
            nc.scalar.activation(out=eca2[:], in_=eca1[:], func=AF.Exp, scale=-1.0)
            nc.vector.tensor_scalar(
                out=attn, in0=eca2[:], scalar1=1.0, scalar2=None, op0=ALU.add
            )
            nc.vector.reciprocal(out=attn[:], in_=attn[:])

        def make_cfb(r0):
            cfb = prod_pool.tile([128, CT * CHUNK], BF16, tag="cf", name="cf")
            for ct in range(CT):
                dst = cfb[:, ct * CHUNK : (ct + 1) * CHUNK]
                nc.scalar.activation(
                    out=dst.rearrange("p (r c) -> p r c", c=W),
                    in_=win_ct(ct, r0), func=AF.Identity,
                    scale=attn[:, ct : ct + 1],
                )
            return cfb

        # ---- main loop over row chunks ----------------------------------
        for ci in range(NCHUNKS):
            r0 = ci * ROWS_PER_CHUNK
            prods = {}
            cfb = None
            for pos, k in enumerate(TAPORD):
                if ci == 0 and pos == 1:
                    pool_reduce(0)
                di, dj = divmod(k, KS)
                pr = prod_pool.tile([128, CT * CHUNK], BF16, tag=f"pr{k}",
                                    name=f"pr{k}")
                fsb = fsb_pool.tile([128, CT * CHUNK], BF16, tag="fsb",
                                    name="fsb")
                for ct in range(CT):
                    mt = pos * CT + ct
                    fp = fps.tile([128, CHUNK], F32, tag="fp", name="fp")
                    for nh in range(NH):
                        for kt in range(CT):
                            lhsT = wf_sb[kt][:, mt * 128 : (mt + 1) * 128]
                            rhs = win_mm(kt, r0 + nh * 8, 8)
                            nc.tensor.matmul(
                                fp[:, nh * 512 : (nh + 1) * 512],
                                lhsT,
                                rhs,
                                start=(kt == 0),
                                stop=(kt == CT - 1),
                            )
                    # scalar-engine evict (+bias, fp32->bf16)
                    nc.scalar.activation(
                        out=fsb[:, ct * CHUNK : (ct + 1) * CHUNK], in_=fp[:],
                        func=AF.Identity, bias=bfp_sb[:, mt : mt + 1],
                        scale=1.0,
                    )
                # one DVE product covers both channel-tiles via strided APs
                nc.vector.tensor_tensor(
                    out=pr.rearrange("p (t r c) -> p t r c", t=CT, c=W),
                    in0=fsb.rearrange("p (t r c) -> p t r c", t=CT, c=W),
                    in1=win2(dj, r0 + di - 1),
                    op=ALU.mult,
                )
                prods[k] = pr

                # interleave adds / attention work as results become available;
                # the tree is arranged so only ONE add remains after the last
                # tap's product (short chunk tail).
                if pos == 2:
                    if ci > 0:
                        cfb = make_cfb(r0)
                elif pos == 3:
                    if ci > 0:
                        nc.vector.tensor_add(prods[0][:], prods[0][:], cfb[:])
                    else:
                        pool_reduce(1)
                elif pos == 4:
                    nc.vector.tensor_add(prods[0][:], prods[0][:], prods[2][:])
                elif pos == 5:
                    nc.vector.tensor_add(prods[4][:], prods[4][:], prods[0][:])
                    if ci == 0:
                        pool_reduce(2)
                elif pos == 6:
                    nc.vector.tensor_add(prods[3][:], prods[3][:], prods[6][:])
                elif pos == 7:
                    if ci == 0:
                        # all center pieces have arrived by now; finish the
                        # pooled sum and the attention on otherwise-idle slack
                        pool_reduce(3)
                        eca_attn()
                        cfb = make_cfb(r0)
                    nc.vector.tensor_add(prods[3][:], prods[3][:], prods[5][:])
                    nc.vector.tensor_add(prods[4][:], prods[4][:], prods[3][:])
                elif pos == 8:
                    nc.vector.tensor_add(prods[4][:], prods[4][:], prods[8][:])

            # projection matmul + y eviction + local BN stats.
            # fused tree result lives in prods[4]; p7 and p1 (ready earliest)
            # join the contraction directly, trading PE cycles for DVE adds.
            # In chunk 0 the attention term joins the contraction too (it is
            # only ready late there).
            mm2_srcs = [prods[7], prods[1], prods[4]]
            if ci == 0:
                mm2_srcs.append(cfb)
            for mt2 in range(CT):
                ypt = [yps.tile([128, 512], F32, tag="yp", name="yp")
                       for _ in range(NH)]
                nsrc = len(mm2_srcs)
                for si, srct in enumerate(mm2_srcs):
                    for kt in range(CT):
                        lhsT2 = wp_sb[kt][:, mt2 * 128 : (mt2 + 1) * 128]
                        for nh in range(NH):
                            nc.tensor.matmul(
                                ypt[nh][:],
                                lhsT2,
                                srct[:, kt * CHUNK + nh * 512 : kt * CHUNK + (nh + 1) * 512],
                                start=(si == 0 and kt == 0),
                                stop=(si == nsrc - 1 and kt == CT - 1),
                            )
                for nh in range(NH):
                    dst = y_sb[mt2][:, r0 * W + nh * 512 : r0 * W + (nh + 1) * 512]
                    # stats straight from PSUM, in parallel with the evict
                    nc.vector.bn_stats(
                        out=stats_sb[mt2][:, ci * NH + nh, :], in_=ypt[nh][:]
                    )
                    nc.scalar.activation(out=dst, in_=ypt[nh][:], func=AF.Copy)

        # ---- global BN stats via all-reduce -----------------------------
        ps = consts.tile([128, 2 * CT], F32, tag="ps", name="ps")
        for mt2 in range(CT):
            mv = consts.tile([128, 2], F32, tag=f"mv{mt2}", name=f"mv{mt2}")
            nc.vector.bn_aggr(out=mv[:], in_=stats_sb[mt2][:])
            mean = mv[:, 0:1]
            var = mv[:, 1:2]
            nc.vector.tensor_scalar(
                out=ps[:, 2 * mt2 : 2 * mt2 + 1], in0=mean, scalar1=float(HW),
                scalar2=None, op0=ALU.mult,
            )
            # sumsq = (var + mean^2) * HW
            nc.vector.scalar_tensor_tensor(
                out=ps[:, 2 * mt2 + 1 : 2 * mt2 + 2], in0=mean, scalar=mean,
                in1=var, op0=ALU.mult, op1=ALU.add,
            )
            nc.vector.tensor_scalar(
                out=ps[:, 2 * mt2 + 1 : 2 * mt2 + 2],
                in0=ps[:, 2 * mt2 + 1 : 2 * mt2 + 2],
                scalar1=float(HW), scalar2=None, op0=ALU.mult,
            )

        # AllGather + a local reduce instead of AllReduce: the gather mesh
        # has fewer synchronization rounds than the reduce+broadcast one.
        ps_b = dram.tile([128, 2 * CT], F32, tag="psb", name="psb")
        gs_b = dram.tile([NCORES, 128, 2 * CT], F32, tag="gsb", name="gsb")
        nc.sync.dma_start(out=ps_b[:], in_=ps[:])
        nc.gpsimd.collective_compute(
            "AllGather",
            ALU.bypass,
            replica_groups=[list(range(NCORES))],
            ins=[ps_b[:].opt()],
            outs=[gs_b[:].opt()],
        )
        # gather output is a buffer-level concatenation: core g's [128, 4]
        # block at flat offset g*512.  DMA it into SBUF with the gather dim
        # innermost, then one reduce sums the 8 blocks.
        gsa = consts.tile([128, 2 * CT, NCORES], F32, tag="gsa", name="gsa")
        nc.scalar.dma_start(
            out=gsa[:], in_=gs_b.rearrange("g p f -> p f g")
        )
        gs = consts.tile([128, 2 * CT], F32, tag="gs", name="gs")
        nc.vector.tensor_reduce(
            out=gs.unsqueeze(2),
            in_=gsa[:],
            axis=mybir.AxisListType.X,
            op=ALU.add,
        )

        # ---- normalize and write out ------------------------------------
        minv = 1.0 / float(B * HW)
        NSL = 4  # normalize/store slices per channel-tile
        SL = HW // NSL
        mg = consts.tile([128, CT], F32, tag="mg", name="mg")
        vg = consts.tile([128, CT], F32, tag="vg", name="vg")
        rr = consts.tile([128, CT], F32, tag="rr", name="rr")
        tt = consts.tile([128, CT], F32, tag="tt", name="tt")
        ac = consts.tile([128, CT], F32, tag="ac", name="ac")
        bc = consts.tile([128, CT], F32, tag="bc", name="bc")
        # mean and E[y^2] (gs columns are [s0, q0, s1, q1])
        gsv = gs.rearrange("p (m two) -> p m two", two=2)
        nc.vector.tensor_scalar(
            out=mg[:], in0=gsv[:, :, 0], scalar1=minv, scalar2=None, op0=ALU.mult
        )
        nc.vector.tensor_scalar(
            out=vg[:], in0=gsv[:, :, 1], scalar1=minv, scalar2=None, op0=ALU.mult
        )
        # vg = E[y^2] - mean^2 + eps  (via -(mean^2 - E[y^2]) + eps)
        nc.vector.tensor_tensor(out=tt[:], in0=mg[:], in1=mg[:], op=ALU.mult)
        nc.vector.tensor_tensor(out=vg[:], in0=tt[:], in1=vg[:], op=ALU.subtract)
        nc.vector.tensor_scalar(
            out=vg[:], in0=vg[:], scalar1=-1.0, scalar2=BN_EPS,
            op0=ALU.mult, op1=ALU.add,
        )
        # rstd via quake seed + 3 Newton iterations, all on the DVE
        vg_i = vg.bitcast(mybir.dt.int32)
        rr_i = rr.bitcast(mybir.dt.int32)
        nc.vector.tensor_scalar(
            out=rr_i[:], in0=vg_i[:], scalar1=1, scalar2=None,
            op0=ALU.arith_shift_right,
        )
        nc.vector.tensor_scalar(
            out=rr_i[:], in0=rr_i[:], scalar1=-1, scalar2=0x5F3759DF,
            op0=ALU.mult, op1=ALU.add,
        )
        hv = consts.tile([128, CT], F32, tag="hv", name="hv")
        nc.vector.tensor_scalar(
            out=hv[:], in0=vg[:], scalar1=0.5, scalar2=None, op0=ALU.mult
        )
        for _ in range(3):
            nc.vector.tensor_tensor(out=tt[:], in0=rr[:], in1=rr[:], op=ALU.mult)
            nc.vector.tensor_tensor(out=tt[:], in0=tt[:], in1=hv[:], op=ALU.mult)
            nc.vector.tensor_scalar(
                out=tt[:], in0=tt[:], scalar1=-1.0, scalar2=1.5,
                op0=ALU.mult, op1=ALU.add,
            )
            nc.vector.tensor_tensor(out=rr[:], in0=rr[:], in1=tt[:], op=ALU.mult)
        # A = rstd * gamma ; bc = beta - mean * A
        nc.vector.tensor_tensor(out=ac[:], in0=rr[:], in1=gam_sb[:], op=ALU.mult)
        nc.vector.tensor_tensor(out=bc[:], in0=mg[:], in1=ac[:], op=ALU.mult)
        nc.vector.tensor_tensor(out=bc[:], in0=bet_sb[:], in1=bc[:], op=ALU.subtract)

        # normalize slices on DVE (5) and scalar (3) in parallel; each
        # slice's output DMA is split in two, spread over two queues that
        # have nothing else to do.
        out_qs = [nc.sync, nc.gpsimd]
        qi = 0
        for s in range(NSL):
            for mt2 in range(CT):
                sl = slice(s * SL, (s + 1) * SL)
                if mt2 == 1 and s >= 1:
                    nc.scalar.activation(
                        out=y_sb[mt2][:, sl], in_=y_sb[mt2][:, sl],
                        func=AF.Identity, bias=bc[:, mt2 : mt2 + 1],
                        scale=ac[:, mt2 : mt2 + 1],
                    )
                else:
                    nc.vector.tensor_scalar(
                        out=y_sb[mt2][:, sl], in0=y_sb[mt2][:, sl],
                        scalar1=ac[:, mt2 : mt2 + 1], scalar2=bc[:, mt2 : mt2 + 1],
                        op0=ALU.mult, op1=ALU.add,
                    )
                for sub in range(2):
                    c0 = s * SL + sub * (SL // 2)
                    c1 = c0 + SL // 2
                    out_qs[qi % len(out_qs)].dma_start(
                        out=yout[mt2, :, c0:c1], in_=y_sb[mt2][:, c0:c1]
                    )
                    qi += 1


_NC = None


def _build_nc(debug=False):
    nc = bacc.Bacc(
        "TRN2", target_bir_lowering=False, debug=debug, num_devices=NCORES
    )
    with tile.TileContext(nc, num_cores=NCORES) as tc:
        _emit(tc)
    nc.compile()
    return nc


def _get_nc():
    global _NC
    if _NC is None:
        _NC = _build_nc()
    return _NC


def _prep_in_maps(x, W_filter, b_filter, w_eca, W_proj, gamma, beta):
    bf = ml_dtypes.bfloat16
    x = np.asarray(x, np.float32)
    W_filter = np.asarray(W_filter, np.float32)
    b_filter = np.asarray(b_filter, np.float32)
    w_eca = np.asarray(w_eca, np.float32)
    W_proj = np.asarray(W_proj, np.float32)
    gamma = np.asarray(gamma, np.float32)
    beta = np.asarray(beta, np.float32)

    # zero-padded layout: 66x66 per channel, image at [1:65, 1:65]
    xpad = np.zeros((B, C, XNR, XROW), np.float32)
    xpad[:, :, 1 : 1 + H, 1 : 1 + W] = x
    xp_h = np.ascontiguousarray(
        xpad.reshape(B, CT, 128, XBUF)
    ).astype(bf)

    # permute mm1 weights to TAPORD tap-major: o' = pos*256 + c
    # (original o = c*9 + k)
    wk = W_filter.reshape(C, KS * KS, C).transpose(1, 0, 2)  # [k, o_c, c]
    wperm = wk[TAPORD].reshape(KS * KS * C, C)
    wf_h = np.ascontiguousarray(wperm.T.reshape(CT, 128, MT1 * 128)).astype(bf)
    bk = b_filter.reshape(C, KS * KS).T  # [k, c]
    bperm = bk[TAPORD].reshape(KS * KS * C)
    bfp_h = np.ascontiguousarray(bperm.reshape(MT1, 128).T).astype(np.float32)

    wp_h = np.ascontiguousarray((0.5 * W_proj).T.reshape(CT, 128, C)).astype(bf)
    weca_h = (w_eca / float(HW)).reshape(1, 3).astype(np.float32)
    gam_h = np.ascontiguousarray(gamma.reshape(CT, 128).T).astype(np.float32)
    bet_h = np.ascontiguousarray(beta.reshape(CT, 128).T).astype(np.float32)

    in_maps = []
    for i in range(B):
        m = {
            "wf": wf_h,
            "bfp": bfp_h,
            "wp": wp_h,
            "weca": weca_h,
            "gam": gam_h,
            "bet": bet_h,
            "xp": xp_h[i],
        }
        in_maps.append(m)
    return in_maps


last_result = None


def kernel(x, W_filter, b_filter, w_eca, W_proj, b_proj, gamma, beta):
    """Full-input, full-output DDF module on 8 NeuronCores."""
    global last_result
    # b_proj is mathematically cancelled by the batch-norm; unused.
    in_maps = _prep_in_maps(x, W_filter, b_filter, w_eca, W_proj, gamma, beta)
    nc = _get_nc()
    trace = bool(int(os.environ.get("DDF_TRACE", "0")))
    res = run_bass_kernel_spmd(nc, in_maps, list(range(NCORES)), trace=trace)
    last_result = res
    out = np.stack(
        [res.results[i]["y"].reshape(C, H, W).astype(np.float32) for i in range(B)]
    )
    return out
